# revision 20
# baseline (speedup 1.0000x reference)
"""InteractionMapInit Trainium2 kernel (v2).

out[i, j, :] = tanh( (X@Wt + bt)[i] - (Dft@Wd + bd)[j] + dnorm[i, j] )  if seg_res[i] == seg_atom[j]
             = 0                                                        otherwise

Block-diagonal over B=8 DT-pairs -> one block per NeuronCore (SPMD), host
scatters blocks into the zeros output.

v2 design (driven by dispatch-cost + roofline analysis):
  - ONE packed fp16 input tensor per core (was 11 tensors) and one fp16
    output tensor: per-exec dispatch cost scales with arg count and bytes.
  - All heavy matmuls in fp16 (full PE rate); distances in fp32 via a
    hi/lo fp16 split of the position tensors (fp16 pairs reconstructed
    on device to fp32 -- keeps D**2 cancellation error ~1e-4).
  - i4 (tiled identity) and r2d (block-broadcast matrix) generated on
    device with gpsimd affine_select instead of DMA'd.
  - Distances computed pre-transposed: D2T[j, i] = rhd.T @ lhd in one
    fp32 matmul; dnorm^T lands directly in the mm2 lhsT tile (no PE
    transposes of dnorm tiles).
  - Output rows trimmed to Ro = roundup(max block rows, 8) <= Rp.
  - tanh on ACT with fp16 output; sqrt/tanh table sets prefetched with
    dummy activations so the ~2.7us loads hide under the preamble.

Per-core device program (Rp padded rows for compute, Ro output rows,
Ap padded atoms):
  tfT  [H, Ro]   = Wt.T @ X.T + (bt - bd)      (fp16 matmuls, fp32 psum)
  df   [Ap, H]   = Dft.T @ Wd                  -> -df flat to r2d row Ap
  D2T  [Ap, Rp]  = rhd.T @ lhd  (fp32, 5-term |t-d|^2 trick), clamp >= 0, sqrt
  dmin/dmax via DVE reduces + one PE transpose; dnorm^T = (DT - dmin)/denom
  lhsT2 [Ap+1, Rp] = [dnorm^T; ones] fp16
  psum[i, (j,h)] = mm1(tfT, i4) + mm2(lhsT2, r2d); tanh -> fp16 -> DMA out
"""

import numpy as np

NR, NA, TD, DD, H, B = 3200, 320, 512, 128, 128, 8
NCORES = 8
P = 128

_last_results = None
_last_nc = None
_last_in_maps = None


def _pack_layout(Rp, Ro, Ap):
    """Two pack regions: A = [128, CA] (col ranges per piece), B = [5, CB]."""
    colsA = [("wd", H), ("dft", Ap), ("biasT", 1), ("wt", 4 * H), ("xt", 4 * Ro)]
    colsB = [("lhd_hi", Rp), ("lhd_lo", Rp), ("rhd_hi", Ap), ("rhd_lo", Ap)]
    layA, c = {}, 0
    for name, n in colsA:
        layA[name] = (c, n)
        c += n
    CA = c
    layB, c = {}, 0
    for name, n in colsB:
        layB[name] = (c, n)
        c += n
    CB = c
    total = P * CA + 5 * CB
    return layA, CA, layB, CB, total


def _host_prep(target_feature, drug_feature, target_pos, drug_pos,
               Wt, bt, Wd, bd, seg_res, seg_atom):
    f32, f16 = np.float32, np.float16
    X = np.asarray(target_feature, f32)
    Dft = np.asarray(drug_feature, f32)
    tp = np.asarray(target_pos, f32)
    dp = np.asarray(drug_pos, f32)
    Wt = np.asarray(Wt, f32)
    Wd = np.asarray(Wd, f32)
    bias = (np.asarray(bt, f32) - np.asarray(bd, f32)).reshape(H)
    seg_res = np.asarray(seg_res)
    seg_atom = np.asarray(seg_atom)

    r0 = np.searchsorted(seg_res, np.arange(B), side="left")
    r1 = np.searchsorted(seg_res, np.arange(B), side="right")
    a0 = np.searchsorted(seg_atom, np.arange(B), side="left")
    a1 = np.searchsorted(seg_atom, np.arange(B), side="right")
    r_cnt = (r1 - r0).astype(int)
    a_cnt = (a1 - a0).astype(int)

    Rp = max(P, int(-(-max(r_cnt) // P)) * P)      # compute row padding
    Ro = max(8, int(-(-max(r_cnt) // 8)) * 8)      # output row padding
    Ap = max(4, int(-(-max(a_cnt) // 4)) * 4)
    assert Ap + 1 <= 128

    layA, CA, layB, CB, total = _pack_layout(Rp, Ro, Ap)
    # wt region A layout: [p, (k, h)] with source row t = k*128 + p
    wt16 = np.ascontiguousarray(
        Wt.astype(f16).reshape(4, P, H).transpose(1, 0, 2).reshape(P, 4 * H))
    wd16 = Wd.astype(f16)
    bias16 = bias.astype(f16)

    def hi_lo(a):
        hi = a.astype(f16)
        lo = (a - hi.astype(f32)).astype(f16)
        return hi, lo

    in_maps = []
    for c in range(B):
        rc, ac = r_cnt[c], a_cnt[c]
        xt = np.zeros((TD, Ro), f32)
        dft = np.zeros((DD, Ap), f32)
        tpp = np.zeros((Rp, 3), f32)
        dpp = np.zeros((Ap, 3), f32)
        xt[:, :rc] = X[r0[c]:r1[c]].T
        tpp[:rc] = tp[r0[c]:r1[c]]
        tpp[rc:] = tp[r1[c] - 1]
        dft[:, :ac] = Dft[a0[c]:a1[c]].T
        dpp[:ac] = dp[a0[c]:a1[c]]
        dpp[ac:] = dp[a1[c] - 1]

        lhd = np.empty((5, Rp), f32)
        lhd[0:3] = tpp.T
        lhd[3] = 1.0
        lhd[4] = (tpp * tpp).sum(axis=1)
        rhd = np.empty((5, Ap), f32)
        rhd[0:3] = -2.0 * dpp.T
        rhd[3] = (dpp * dpp).sum(axis=1)
        rhd[4] = 1.0
        lhd_hi, lhd_lo = hi_lo(lhd)
        rhd_hi, rhd_lo = hi_lo(rhd)

        pack = np.empty(total, f16)
        A = pack[:P * CA].reshape(P, CA)
        Bv = pack[P * CA:].reshape(5, CB)

        def putA(name, arr):
            off, n = layA[name]
            A[:, off:off + n] = np.asarray(arr, f16).reshape(P, n)

        def putB(name, arr):
            off, n = layB[name]
            Bv[:, off:off + n] = np.asarray(arr, f16).reshape(5, n)

        putA("wd", wd16)
        putA("dft", dft)
        putA("biasT", bias16.reshape(P, 1))
        putA("wt", wt16)
        putA("xt", np.ascontiguousarray(
            xt.reshape(4, P, Ro).transpose(1, 0, 2).reshape(P, 4 * Ro)))
        putB("lhd_hi", lhd_hi)
        putB("lhd_lo", lhd_lo)
        putB("rhd_hi", rhd_hi)
        putB("rhd_lo", rhd_lo)
        in_maps.append({"pack": pack})

    meta = dict(r0=r0, a0=a0, r_cnt=r_cnt, a_cnt=a_cnt, Rp=Rp, Ro=Ro, Ap=Ap)
    return in_maps, meta


def build_bass(Rp, Ro, Ap):
    from contextlib import ExitStack

    import concourse.bacc as bacc
    import concourse.bass_isa as bass_isa
    import concourse.mybir as mybir
    import concourse.tile as tile
    from concourse.masks import make_identity

    F32 = mybir.dt.float32
    F32R = mybir.dt.float32r
    F16 = mybir.dt.float16
    AX = mybir.AxisListType
    OP = mybir.AluOpType
    AF = mybir.ActivationFunctionType

    K_TD = TD // P        # 4 contraction chunks for the target linear
    RT = Rp // P          # 128-row tiles
    AH = Ap * H
    NCH = AH // 512       # 512-wide psum chunks (4 atoms x H)
    GRP = 4               # chunks per psum group (4 banks; x2 groups = 8)

    layA, CA, layB, CB, total = _pack_layout(Rp, Ro, Ap)

    nc = bacc.Bacc("TRN2", target_bir_lowering=False, debug=False,
                   num_devices=NCORES, enable_partition_id=False)

    pack_d = nc.dram_tensor("pack", [total], F16, kind="ExternalInput").ap()
    out_d = nc.dram_tensor("out", [Ro, AH], F16, kind="ExternalOutput").ap()

    with tile.TileContext(nc) as tc, ExitStack() as ctx:
        singles = ctx.enter_context(tc.tile_pool(name="singles", bufs=1))
        temps = ctx.enter_context(tc.tile_pool(name="temps", bufs=2))
        psum = ctx.enter_context(tc.tile_pool(name="psum", bufs=2, space="PSUM"))
        outs = ctx.enter_context(tc.tile_pool(name="outs", bufs=3))
        dram = ctx.enter_context(tc.tile_pool(name="dram", bufs=1, space="DRAM"))

        # ---------------- inputs to SBUF: 3 DMAs on 3 queues ----------------
        # B (positions) first: it heads the serial distance chain.
        packB = singles.tile([5, CB], F16, name="packB")
        nc.sync.dma_start(out=packB,
                          in_=pack_d[P * CA:].rearrange("(p c) -> p c", p=5))
        packA = singles.tile([P, CA], F16, name="packA")
        offA2, _ = layA["xt"]
        nc.scalar.dma_start(
            out=packA[:, :offA2],
            in_=pack_d[:P * CA].rearrange("(p c) -> p c", p=P)[:, :offA2])
        nc.gpsimd.dma_start(
            out=packA[:, offA2:],
            in_=pack_d[:P * CA].rearrange("(p c) -> p c", p=P)[:, offA2:])

        def pA(name):
            off, n = layA[name]
            return packA[:, off:off + n]

        def pB(name):
            off, n = layB[name]
            return packB[:, off:off + n]

        wd_sb = pA("wd")
        dft_sb = pA("dft")
        biasT = pA("biasT")
        wt_sb = pA("wt").rearrange("p (k h) -> p k h", h=H)
        xt_sb = pA("xt").rearrange("p (k i) -> p k i", i=Ro)
        lhd_hi = pB("lhd_hi")
        lhd_lo = pB("lhd_lo")
        rhd_hi = pB("rhd_hi")
        rhd_lo = pB("rhd_lo")

        # ---------------- on-device constants (cheap, off critical path) ----
        idn16 = singles.tile([P, P], F16, name="idn16")
        nc.gpsimd.memset(idn16, 0.0)
        nc.gpsimd.affine_select(
            out=idn16, in_=idn16, compare_op=OP.not_equal, fill=1.0, base=0,
            pattern=[[-1, P]], channel_multiplier=1)
        # mm1 rhs: rhs[h', (j, h)] = idn16[h', h]  (j is a stride-0 dim)
        i4_bc = idn16.rearrange("p (one h) -> p one h", one=1).broadcast_to([P, 4, P])

        # I48ext [Ap+1, Ap]: identity in rows 0..Ap-1, zero row Ap
        i48 = singles.tile([Ap + 1, Ap], F16, name="i48")
        nc.gpsimd.memset(i48, 0.0)
        nc.gpsimd.affine_select(
            out=i48[:Ap], in_=i48[:Ap], compare_op=OP.not_equal, fill=1.0, base=0,
            pattern=[[-1, Ap]], channel_multiplier=1)

        # r2d rows 0..Ap-1: r2d[j, (j',h)] = (j == j') via DVE broadcast copy
        # (split into halves, issued later to keep the distance chain head of
        # the DVE queue); row Ap: -df flat (DMA roundtrip below)
        r2d = singles.tile([Ap + 1, AH], F16, name="r2d")

        def r2d_part(q, nq):
            jc = Ap // nq
            j0 = q * jc
            nc.vector.tensor_copy(
                out=r2d[:Ap, j0 * H:(j0 + jc) * H].rearrange(
                    "j (jp h) -> j jp h", h=H),
                in_=i48[:Ap, j0:j0 + jc].rearrange(
                    "j (jp one) -> j jp one", one=1).broadcast_to([Ap, jc, H]))

        # ---------------- ACT table prefetch ----------------
        ones_sb = singles.tile([1, 64], F32, name="ones_sb")
        nc.vector.memset(ones_sb, 1.0)
        scr = temps.tile([1, 16], F32, name="scr")
        nc.scalar.activation(out=scr, in_=ones_sb[:, :16], func=AF.Sqrt)

        # lhsT2: rows 0..Ap-1 get dnorm^T below; row Ap stays all-ones
        lhsT2 = singles.tile([Ap + 1, Rp], F16, name="lhsT2")
        nc.vector.memset(lhsT2[32:Ap + 1, :], 1.0)

        # ---------------- distances first (head of the serial chain) ----------
        biasT32 = temps.tile([P, 1], F32, name="biasT32")
        nc.vector.tensor_copy(out=biasT32, in_=biasT)
        lhd_sb = temps.tile([5, Rp], F32R, name="lhd_sb")
        rhd_sb = temps.tile([5, Ap], F32R, name="rhd_sb")
        ps_d = psum.tile([P, GRP * 512], F32, tag="ps", name="ps_d")
        with tc.high_priority():
            nc.vector.tensor_tensor(out=lhd_sb, in0=lhd_hi, in1=lhd_lo, op=OP.add)
            nc.vector.tensor_tensor(out=rhd_sb, in0=rhd_hi, in1=rhd_lo, op=OP.add)
            nc.tensor.matmul(ps_d[:Ap, :Rp], lhsT=rhd_sb, rhs=lhd_sb,
                             start=True, stop=True)

        # ---------------- df = Dft.T @ Wd ; -df -> r2d row Ap ----------------
        ps_df = psum.tile([P, GRP * 512], F32, tag="ps", name="ps_df")
        nc.tensor.matmul(ps_df[:Ap, :H], lhsT=dft_sb, rhs=wd_sb,
                         start=True, stop=True)

        # ---------------- tfT = Wt.T @ X.T + bias  [H, Ro] ----------------
        ps_tf = psum.tile([P, GRP * 512], F32, tag="ps", name="ps_tf")
        for k in range(K_TD):
            nc.tensor.matmul(ps_tf[:, :Ro], lhsT=wt_sb[:, k, :],
                             rhs=xt_sb[:, k, :], start=(k == 0), stop=(k == K_TD - 1))

        # -df roundtrip gates the first main-loop group: keep it hot and put
        # the two hops on different queues so their issue costs overlap
        dfneg = temps.tile([Ap, H], F16, name="dfneg")
        dscr = dram.tile([AH], F16, name="dscr")
        with tc.high_priority():
            nc.vector.tensor_scalar_mul(dfneg, ps_df[:Ap, :H], -1.0)
            nc.gpsimd.dma_start(out=dscr.rearrange("(a h) -> a h", h=H), in_=dfneg)
            nc.sync.dma_start(out=r2d[Ap:Ap + 1, :], in_=dscr[None, :])

        r2d_part(0, 4)
        r2d_part(1, 4)

        dt2 = temps.tile([Ap, Rp], F32, name="dt2")
        dt = singles.tile([Ap, Rp], F32, name="dt")
        with tc.high_priority():
            # clamp >= 0 via ACT Relu (same table set as Sqrt, reads PSUM
            # directly) -- keeps the clamp->sqrt hop on one engine, off DVE
            nc.scalar.activation(out=dt2, in_=ps_d[:Ap, :Rp], func=AF.Relu)
            nc.scalar.activation(out=dt, in_=dt2, func=AF.Sqrt)

        tfT = singles.tile([P, Rp], F16, name="tfT")
        if Ro < Rp:
            nc.vector.memset(tfT[:, Ro:], 0.0)
        nc.vector.tensor_scalar(out=tfT[:, :Ro], in0=ps_tf[:, :Ro], scalar1=biasT32,
                                scalar2=None, op0=OP.add)
        # prefetch tanh table now (only remaining ACT set)
        scr2 = temps.tile([1, 16], F32, name="scr2")
        nc.scalar.activation(out=scr2, in_=dt[:1, :16], func=AF.Tanh)

        # per-block dmin/dmax: row stats as [-rowmin, rowmax], then one gpsimd
        # all-reduce(max) across partitions gives [-dmin, dmax] on EVERY
        # partition -- no PE transposes, no broadcast-back matmul.
        stats = temps.tile([Ap, 2], F32, name="stats")
        cols = temps.tile([Ap, 2], F32, name="cols")  # [-dmin, dmax] per row
        diff48 = temps.tile([Ap, 1], F32, name="diff48")
        denom48 = temps.tile([Ap, 1], F32, name="denom48")
        inv48 = temps.tile([Ap, 1], F32, name="inv48")
        with tc.high_priority():
            nc.vector.tensor_reduce(out=stats[:, 0:1], in_=dt, axis=AX.X, op=OP.min,
                                    negate=True)
            nc.vector.tensor_reduce(out=stats[:, 1:2], in_=dt, axis=AX.X, op=OP.max)
            nc.gpsimd.partition_all_reduce(cols, stats, channels=Ap,
                                           reduce_op=bass_isa.ReduceOp.max)
            # diff = dmax - dmin = cols1 + cols0
            nc.vector.tensor_scalar(out=diff48, in0=cols[:, 1:2],
                                    scalar1=cols[:, 0:1], scalar2=None, op0=OP.add)
            nc.vector.tensor_scalar_max(denom48, diff48, 1e-30)
            nc.vector.reciprocal(out=inv48, in_=denom48)
            # dnorm^T = (dt + (-dmin)) * inv straight into lhsT2 rows 0..Ap-1
            nc.vector.tensor_scalar(out=lhsT2[:Ap, :], in0=dt,
                                    scalar1=cols[:, 0:1], scalar2=inv48,
                                    op0=OP.add, op1=OP.mult)
        for _q in range(2, 4):
            r2d_part(_q, 4)

        # ---------------- main: psum = tf - df + dnorm ; tanh ; out ----------------
        def group_sizes(rt):
            n = NCH
            if rt == 0 and n > 1:          # small first group: start ACT sooner
                rest = n - 1
                sizes = [1]
            elif rt == RT - 1 and n > 1:   # small last group: drain sooner
                rest = n - 1
                sizes = []
            else:
                rest = n
                sizes = []
            while rest > 0:
                take = min(GRP, rest)
                sizes.append(take)
                rest -= take
            if rt == RT - 1 and n > 1:
                sizes.append(1)
            return sizes

        gi = 0
        for rt in range(RT):
            i_lo = rt * P
            m = min(Ro, i_lo + P) - i_lo          # output rows this tile
            if m <= 0:
                break
            l2_sl = lhsT2[:, i_lo:i_lo + P]
            tf_sl = tfT[:, i_lo:i_lo + P]
            ch0 = 0
            for g in group_sizes(rt):
                gw = 512 * g
                pso = psum.tile([P, GRP * 512], F32, tag="ps", name="pso")
                for c in range(g):
                    ch = ch0 + c
                    csl = slice(512 * c, 512 * (c + 1))
                    nc.tensor.matmul(pso[:, csl], lhsT=tf_sl, rhs=i4_bc,
                                     start=True, stop=False)
                    nc.tensor.matmul(pso[:, csl], lhsT=l2_sl,
                                     rhs=r2d[:, 512 * ch:512 * (ch + 1)],
                                     start=False, stop=True)
                ob = outs.tile([P, GRP * 512], F16, name="ob")
                nc.scalar.activation(out=ob[:, :gw], in_=pso[:, :gw], func=AF.Tanh)
                eng = nc.sync if gi % 2 == 0 else nc.gpsimd
                eng.dma_start(
                    out=out_d[i_lo:i_lo + m, 512 * ch0:512 * ch0 + gw],
                    in_=ob[:m, :gw])
                ch0 += g
                gi += 1

    nc.compile()
    return nc


def kernel(**inputs) -> np.ndarray:
    global _last_results, _last_nc, _last_in_maps
    import os
    if os.environ.get("BASS_TRACE") and not os.environ.get("BASS_NEVER_TRACE"):
        try:
            import antenv.axon_hooks  # noqa: F401  (NTFF profile hook)
        except ImportError:
            # Tracing is requested but the axon NTFF hook is absent in this
            # container; run untraced instead of crashing.
            os.environ["BASS_NEVER_TRACE"] = "1"

    in_maps, meta = _host_prep(**inputs)
    Rp, Ro, Ap = meta["Rp"], meta["Ro"], meta["Ap"]

    nc = build_bass(Rp, Ro, Ap)
    _last_nc, _last_in_maps = nc, in_maps

    from concourse.bass_utils import run_bass_kernel_spmd
    res = run_bass_kernel_spmd(nc, in_maps, core_ids=list(range(NCORES)))
    _last_results = res

    out = np.zeros((NR, NA, H), np.float32)
    for c in range(B):
        rc, ac = int(meta["r_cnt"][c]), int(meta["a_cnt"][c])
        if rc == 0 or ac == 0:
            continue
        blk = res.results[c]["out"].reshape(Ro, Ap, H)
        r0, a0 = int(meta["r0"][c]), int(meta["a0"][c])
        out[r0:r0 + rc, a0:a0 + ac, :] = blk[:rc, :ac, :].astype(np.float32)
    return out


# revision 21
# speedup vs baseline: 6.9990x; 6.9990x over previous
"""InteractionMapInit Trainium2 kernel (v2).

out[i, j, :] = tanh( (X@Wt + bt)[i] - (Dft@Wd + bd)[j] + dnorm[i, j] )  if seg_res[i] == seg_atom[j]
             = 0                                                        otherwise

Block-diagonal over B=8 DT-pairs -> one block per NeuronCore (SPMD), host
scatters blocks into the zeros output.

v2 design (driven by dispatch-cost + roofline analysis):
  - ONE packed fp16 input tensor per core (was 11 tensors) and one fp16
    output tensor: per-exec dispatch cost scales with arg count and bytes.
  - All heavy matmuls in fp16 (full PE rate); distances in fp32 via a
    hi/lo fp16 split of the position tensors (fp16 pairs reconstructed
    on device to fp32 -- keeps D**2 cancellation error ~1e-4).
  - i4 (tiled identity) and r2d (block-broadcast matrix) generated on
    device with gpsimd affine_select instead of DMA'd.
  - Distances computed pre-transposed: D2T[j, i] = rhd.T @ lhd in one
    fp32 matmul; dnorm^T lands directly in the mm2 lhsT tile (no PE
    transposes of dnorm tiles).
  - Output rows trimmed to Ro = roundup(max block rows, 8) <= Rp.
  - tanh on ACT with fp16 output; sqrt/tanh table sets prefetched with
    dummy activations so the ~2.7us loads hide under the preamble.

Per-core device program (Rp padded rows for compute, Ro output rows,
Ap padded atoms):
  tfT  [H, Ro]   = Wt.T @ X.T + (bt - bd)      (fp16 matmuls, fp32 psum)
  df   [Ap, H]   = Dft.T @ Wd                  -> -df flat to r2d row Ap
  D2T  [Ap, Rp]  = rhd.T @ lhd  (fp32, 5-term |t-d|^2 trick), clamp >= 0, sqrt
  dmin/dmax via DVE reduces + one PE transpose; dnorm^T = (DT - dmin)/denom
  lhsT2 [Ap+1, Rp] = [dnorm^T; ones] fp16
  psum[i, (j,h)] = mm1(tfT, i4) + mm2(lhsT2, r2d); tanh -> fp16 -> DMA out
"""

import numpy as np

NR, NA, TD, DD, H, B = 3200, 320, 512, 128, 128, 8
NCORES = 8
P = 128

_last_results = None
_last_nc = None
_last_in_maps = None


def _pack_layout(Rp, Ro, Ap):
    """Two pack regions: A = [128, CA] (col ranges per piece), B = [5, CB]."""
    colsA = [("wd", H), ("dft", Ap), ("biasT", 1), ("wt", 4 * H), ("xt", 4 * Ro)]
    colsB = [("lhd_hi", Rp), ("lhd_lo", Rp), ("rhd_hi", Ap), ("rhd_lo", Ap)]
    layA, c = {}, 0
    for name, n in colsA:
        layA[name] = (c, n)
        c += n
    CA = c
    layB, c = {}, 0
    for name, n in colsB:
        layB[name] = (c, n)
        c += n
    CB = c
    total = P * CA + 5 * CB
    return layA, CA, layB, CB, total


def _host_prep(target_feature, drug_feature, target_pos, drug_pos,
               Wt, bt, Wd, bd, seg_res, seg_atom):
    f32, f16 = np.float32, np.float16
    X = np.asarray(target_feature, f32)
    Dft = np.asarray(drug_feature, f32)
    tp = np.asarray(target_pos, f32)
    dp = np.asarray(drug_pos, f32)
    Wt = np.asarray(Wt, f32)
    Wd = np.asarray(Wd, f32)
    bias = (np.asarray(bt, f32) - np.asarray(bd, f32)).reshape(H)
    seg_res = np.asarray(seg_res)
    seg_atom = np.asarray(seg_atom)

    r0 = np.searchsorted(seg_res, np.arange(B), side="left")
    r1 = np.searchsorted(seg_res, np.arange(B), side="right")
    a0 = np.searchsorted(seg_atom, np.arange(B), side="left")
    a1 = np.searchsorted(seg_atom, np.arange(B), side="right")
    r_cnt = (r1 - r0).astype(int)
    a_cnt = (a1 - a0).astype(int)

    Rp = max(P, int(-(-max(r_cnt) // P)) * P)      # compute row padding
    Ro = max(8, int(-(-max(r_cnt) // 8)) * 8)      # output row padding
    Ap = max(4, int(-(-max(a_cnt) // 4)) * 4)
    assert Ap + 1 <= 128

    layA, CA, layB, CB, total = _pack_layout(Rp, Ro, Ap)
    # wt region A layout: [p, (k, h)] with source row t = k*128 + p
    wt16 = np.ascontiguousarray(
        Wt.astype(f16).reshape(4, P, H).transpose(1, 0, 2).reshape(P, 4 * H))
    wd16 = Wd.astype(f16)
    bias16 = bias.astype(f16)

    def hi_lo(a):
        hi = a.astype(f16)
        lo = (a - hi.astype(f32)).astype(f16)
        return hi, lo

    in_maps = []
    for c in range(B):
        rc, ac = r_cnt[c], a_cnt[c]
        xt = np.zeros((TD, Ro), f32)
        dft = np.zeros((DD, Ap), f32)
        tpp = np.zeros((Rp, 3), f32)
        dpp = np.zeros((Ap, 3), f32)
        xt[:, :rc] = X[r0[c]:r1[c]].T
        tpp[:rc] = tp[r0[c]:r1[c]]
        tpp[rc:] = tp[r1[c] - 1]
        dft[:, :ac] = Dft[a0[c]:a1[c]].T
        dpp[:ac] = dp[a0[c]:a1[c]]
        dpp[ac:] = dp[a1[c] - 1]

        lhd = np.empty((5, Rp), f32)
        lhd[0:3] = tpp.T
        lhd[3] = 1.0
        lhd[4] = (tpp * tpp).sum(axis=1)
        rhd = np.empty((5, Ap), f32)
        rhd[0:3] = -2.0 * dpp.T
        rhd[3] = (dpp * dpp).sum(axis=1)
        rhd[4] = 1.0
        lhd_hi, lhd_lo = hi_lo(lhd)
        rhd_hi, rhd_lo = hi_lo(rhd)

        pack = np.empty(total, f16)
        A = pack[:P * CA].reshape(P, CA)
        Bv = pack[P * CA:].reshape(5, CB)

        def putA(name, arr):
            off, n = layA[name]
            A[:, off:off + n] = np.asarray(arr, f16).reshape(P, n)

        def putB(name, arr):
            off, n = layB[name]
            Bv[:, off:off + n] = np.asarray(arr, f16).reshape(5, n)

        putA("wd", wd16)
        putA("dft", dft)
        putA("biasT", bias16.reshape(P, 1))
        putA("wt", wt16)
        putA("xt", np.ascontiguousarray(
            xt.reshape(4, P, Ro).transpose(1, 0, 2).reshape(P, 4 * Ro)))
        putB("lhd_hi", lhd_hi)
        putB("lhd_lo", lhd_lo)
        putB("rhd_hi", rhd_hi)
        putB("rhd_lo", rhd_lo)
        in_maps.append({"pack": pack})

    meta = dict(r0=r0, a0=a0, r_cnt=r_cnt, a_cnt=a_cnt, Rp=Rp, Ro=Ro, Ap=Ap)
    return in_maps, meta


def build_bass(Rp, Ro, Ap):
    from contextlib import ExitStack

    import concourse.bacc as bacc
    import concourse.bass_isa as bass_isa
    import concourse.mybir as mybir
    import concourse.tile as tile
    from concourse.masks import make_identity

    F32 = mybir.dt.float32
    F32R = mybir.dt.float32r
    F16 = mybir.dt.float16
    AX = mybir.AxisListType
    OP = mybir.AluOpType
    AF = mybir.ActivationFunctionType

    K_TD = TD // P        # 4 contraction chunks for the target linear
    RT = Rp // P          # 128-row tiles
    AH = Ap * H
    NCH = AH // 512       # 512-wide psum chunks (4 atoms x H)
    GRP = 4               # chunks per psum group (4 banks; x2 groups = 8)

    layA, CA, layB, CB, total = _pack_layout(Rp, Ro, Ap)

    nc = bacc.Bacc("TRN2", target_bir_lowering=False, debug=False,
                   num_devices=NCORES, enable_partition_id=False)

    pack_d = nc.dram_tensor("pack", [total], F16, kind="ExternalInput").ap()
    out_d = nc.dram_tensor("out", [Ro, AH], F16, kind="ExternalOutput").ap()

    with tile.TileContext(nc) as tc, ExitStack() as ctx:
        singles = ctx.enter_context(tc.tile_pool(name="singles", bufs=1))
        temps = ctx.enter_context(tc.tile_pool(name="temps", bufs=2))
        psum = ctx.enter_context(tc.tile_pool(name="psum", bufs=2, space="PSUM"))
        outs = ctx.enter_context(tc.tile_pool(name="outs", bufs=4))
        dram = ctx.enter_context(tc.tile_pool(name="dram", bufs=1, space="DRAM"))

        # ---------------- inputs to SBUF: 3 DMAs on 3 queues ----------------
        # B (positions) first: it heads the serial distance chain.
        packB = singles.tile([5, CB], F16, name="packB")
        nc.sync.dma_start(out=packB,
                          in_=pack_d[P * CA:].rearrange("(p c) -> p c", p=5))
        packA = singles.tile([P, CA], F16, name="packA")
        offA2, _ = layA["xt"]
        nc.scalar.dma_start(
            out=packA[:, :offA2],
            in_=pack_d[:P * CA].rearrange("(p c) -> p c", p=P)[:, :offA2])
        nc.gpsimd.dma_start(
            out=packA[:, offA2:],
            in_=pack_d[:P * CA].rearrange("(p c) -> p c", p=P)[:, offA2:])

        def pA(name):
            off, n = layA[name]
            return packA[:, off:off + n]

        def pB(name):
            off, n = layB[name]
            return packB[:, off:off + n]

        wd_sb = pA("wd")
        dft_sb = pA("dft")
        biasT = pA("biasT")
        wt_sb = pA("wt").rearrange("p (k h) -> p k h", h=H)
        xt_sb = pA("xt").rearrange("p (k i) -> p k i", i=Ro)
        lhd_hi = pB("lhd_hi")
        lhd_lo = pB("lhd_lo")
        rhd_hi = pB("rhd_hi")
        rhd_lo = pB("rhd_lo")

        # ---------------- on-device constants (cheap, off critical path) ----
        idn16 = singles.tile([P, P], F16, name="idn16")
        nc.gpsimd.memset(idn16, 0.0)
        nc.gpsimd.affine_select(
            out=idn16, in_=idn16, compare_op=OP.not_equal, fill=1.0, base=0,
            pattern=[[-1, P]], channel_multiplier=1)
        # mm1 rhs: rhs[h', (j, h)] = idn16[h', h]  (j is a stride-0 dim)
        i4_bc = idn16.rearrange("p (one h) -> p one h", one=1).broadcast_to([P, 4, P])

        # I48ext [Ap+1, Ap]: identity in rows 0..Ap-1, zero row Ap
        i48 = singles.tile([Ap + 1, Ap], F16, name="i48")
        nc.gpsimd.memset(i48, 0.0)
        nc.gpsimd.affine_select(
            out=i48[:Ap], in_=i48[:Ap], compare_op=OP.not_equal, fill=1.0, base=0,
            pattern=[[-1, Ap]], channel_multiplier=1)

        # r2d rows 0..Ap-1: r2d[j, (j',h)] = (j == j') via DVE broadcast copy
        # (split into halves, issued later to keep the distance chain head of
        # the DVE queue); row Ap: -df flat (DMA roundtrip below)
        r2d = singles.tile([Ap + 1, AH], F16, name="r2d")

        def r2d_part(q, nq):
            jc = Ap // nq
            j0 = q * jc
            nc.vector.tensor_copy(
                out=r2d[:Ap, j0 * H:(j0 + jc) * H].rearrange(
                    "j (jp h) -> j jp h", h=H),
                in_=i48[:Ap, j0:j0 + jc].rearrange(
                    "j (jp one) -> j jp one", one=1).broadcast_to([Ap, jc, H]))

        # ---------------- ACT table prefetch ----------------
        ones_sb = singles.tile([1, 64], F32, name="ones_sb")
        nc.vector.memset(ones_sb, 1.0)
        scr = temps.tile([1, 16], F32, name="scr")
        nc.scalar.activation(out=scr, in_=ones_sb[:, :16], func=AF.Sqrt)

        # lhsT2: rows 0..Ap-1 get dnorm^T below; row Ap stays all-ones
        lhsT2 = singles.tile([Ap + 1, Rp], F16, name="lhsT2")
        nc.vector.memset(lhsT2[32:Ap + 1, :], 1.0)

        # ---------------- distances first (head of the serial chain) ----------
        biasT32 = temps.tile([P, 1], F32, name="biasT32")
        nc.vector.tensor_copy(out=biasT32, in_=biasT)
        lhd_sb = temps.tile([5, Rp], F32R, name="lhd_sb")
        rhd_sb = temps.tile([5, Ap], F32R, name="rhd_sb")
        ps_d = psum.tile([P, GRP * 512], F32, tag="ps", name="ps_d")
        with tc.high_priority():
            nc.vector.tensor_tensor(out=lhd_sb, in0=lhd_hi, in1=lhd_lo, op=OP.add)
            nc.vector.tensor_tensor(out=rhd_sb, in0=rhd_hi, in1=rhd_lo, op=OP.add)
            nc.tensor.matmul(ps_d[:Ap, :Rp], lhsT=rhd_sb, rhs=lhd_sb,
                             start=True, stop=True)

        # ---------------- df = Dft.T @ Wd ; -df -> r2d row Ap ----------------
        ps_df = psum.tile([P, GRP * 512], F32, tag="ps", name="ps_df")
        nc.tensor.matmul(ps_df[:Ap, :H], lhsT=dft_sb, rhs=wd_sb,
                         start=True, stop=True)

        # ---------------- tfT = Wt.T @ X.T + bias  [H, Ro] ----------------
        ps_tf = psum.tile([P, GRP * 512], F32, tag="ps", name="ps_tf")
        for k in range(K_TD):
            nc.tensor.matmul(ps_tf[:, :Ro], lhsT=wt_sb[:, k, :],
                             rhs=xt_sb[:, k, :], start=(k == 0), stop=(k == K_TD - 1))

        # -df roundtrip gates the first main-loop group: keep it hot and put
        # the two hops on different queues so their issue costs overlap
        dfneg = temps.tile([Ap, H], F16, name="dfneg")
        dscr = dram.tile([AH], F16, name="dscr")
        with tc.high_priority():
            nc.vector.tensor_scalar_mul(dfneg, ps_df[:Ap, :H], -1.0)
            nc.gpsimd.dma_start(out=dscr.rearrange("(a h) -> a h", h=H), in_=dfneg)
            nc.sync.dma_start(out=r2d[Ap:Ap + 1, :], in_=dscr[None, :])

        r2d_part(0, 4)
        r2d_part(1, 4)

        dt2 = temps.tile([Ap, Rp], F32, name="dt2")
        dt = singles.tile([Ap, Rp], F32, name="dt")
        with tc.high_priority():
            # clamp >= 0 via ACT Relu (same table set as Sqrt, reads PSUM
            # directly) -- keeps the clamp->sqrt hop on one engine, off DVE
            nc.scalar.activation(out=dt2, in_=ps_d[:Ap, :Rp], func=AF.Relu)
            nc.scalar.activation(out=dt, in_=dt2, func=AF.Sqrt)

        tfT = singles.tile([P, Rp], F16, name="tfT")
        if Ro < Rp:
            nc.vector.memset(tfT[:, Ro:], 0.0)
        nc.vector.tensor_scalar(out=tfT[:, :Ro], in0=ps_tf[:, :Ro], scalar1=biasT32,
                                scalar2=None, op0=OP.add)
        # prefetch tanh table now (only remaining ACT set)
        scr2 = temps.tile([1, 16], F32, name="scr2")
        nc.scalar.activation(out=scr2, in_=dt[:1, :16], func=AF.Tanh)

        # per-block dmin/dmax: row stats as [-rowmin, rowmax], then one gpsimd
        # all-reduce(max) across partitions gives [-dmin, dmax] on EVERY
        # partition -- no PE transposes, no broadcast-back matmul.
        stats = temps.tile([Ap, 2], F32, name="stats")
        cols = temps.tile([Ap, 2], F32, name="cols")  # [-dmin, dmax] per row
        diff48 = temps.tile([Ap, 1], F32, name="diff48")
        denom48 = temps.tile([Ap, 1], F32, name="denom48")
        inv48 = temps.tile([Ap, 1], F32, name="inv48")
        with tc.high_priority():
            nc.vector.tensor_reduce(out=stats[:, 0:1], in_=dt, axis=AX.X, op=OP.min,
                                    negate=True)
            nc.vector.tensor_reduce(out=stats[:, 1:2], in_=dt, axis=AX.X, op=OP.max)
            nc.gpsimd.partition_all_reduce(cols, stats, channels=Ap,
                                           reduce_op=bass_isa.ReduceOp.max)
            # diff = dmax - dmin = cols1 + cols0
            nc.vector.tensor_scalar(out=diff48, in0=cols[:, 1:2],
                                    scalar1=cols[:, 0:1], scalar2=None, op0=OP.add)
            nc.vector.tensor_scalar_max(denom48, diff48, 1e-30)
            nc.vector.reciprocal(out=inv48, in_=denom48)
            # dnorm^T = (dt + (-dmin)) * inv straight into lhsT2 rows 0..Ap-1
            nc.vector.tensor_scalar(out=lhsT2[:Ap, :], in0=dt,
                                    scalar1=cols[:, 0:1], scalar2=inv48,
                                    op0=OP.add, op1=OP.mult)
        for _q in range(2, 4):
            r2d_part(_q, 4)

        # ---------------- main: psum = tf - df + dnorm ; tanh ; out ----------------
        def group_sizes(rt):
            n = NCH
            if rt == 0 and n > 1:          # small first group: start ACT sooner
                rest = n - 1
                sizes = [1]
            elif rt == RT - 1 and n > 1:   # small last group: drain sooner
                rest = n - 1
                sizes = []
            else:
                rest = n
                sizes = []
            while rest > 0:
                take = min(GRP, rest)
                sizes.append(take)
                rest -= take
            if rt == RT - 1 and n > 1:
                sizes.append(1)
            return sizes

        gi = 0
        for rt in range(RT):
            i_lo = rt * P
            m = min(Ro, i_lo + P) - i_lo          # output rows this tile
            if m <= 0:
                break
            l2_sl = lhsT2[:, i_lo:i_lo + P]
            tf_sl = tfT[:, i_lo:i_lo + P]
            ch0 = 0
            for g in group_sizes(rt):
                gw = 512 * g
                pso = psum.tile([P, GRP * 512], F32, tag="ps", name="pso")
                for c in range(g):
                    ch = ch0 + c
                    csl = slice(512 * c, 512 * (c + 1))
                    nc.tensor.matmul(pso[:, csl], lhsT=tf_sl, rhs=i4_bc,
                                     start=True, stop=False)
                    nc.tensor.matmul(pso[:, csl], lhsT=l2_sl,
                                     rhs=r2d[:, 512 * ch:512 * (ch + 1)],
                                     start=False, stop=True)
                ob = outs.tile([P, GRP * 512], F16, name="ob")
                nc.scalar.activation(out=ob[:, :gw], in_=pso[:, :gw], func=AF.Tanh)
                eng = (nc.sync, nc.gpsimd, nc.scalar)[gi % 3]
                eng.dma_start(
                    out=out_d[i_lo:i_lo + m, 512 * ch0:512 * ch0 + gw],
                    in_=ob[:m, :gw])
                ch0 += g
                gi += 1

    nc.compile()
    return nc


def kernel(**inputs) -> np.ndarray:
    global _last_results, _last_nc, _last_in_maps
    import os
    if os.environ.get("BASS_TRACE") and not os.environ.get("BASS_NEVER_TRACE"):
        try:
            import antenv.axon_hooks  # noqa: F401  (NTFF profile hook)
        except ImportError:
            # Tracing is requested but the axon NTFF hook is absent in this
            # container; run untraced instead of crashing.
            os.environ["BASS_NEVER_TRACE"] = "1"

    in_maps, meta = _host_prep(**inputs)
    Rp, Ro, Ap = meta["Rp"], meta["Ro"], meta["Ap"]

    nc = build_bass(Rp, Ro, Ap)
    _last_nc, _last_in_maps = nc, in_maps

    from concourse.bass_utils import run_bass_kernel_spmd
    res = run_bass_kernel_spmd(nc, in_maps, core_ids=list(range(NCORES)))
    _last_results = res

    out = np.zeros((NR, NA, H), np.float32)
    for c in range(B):
        rc, ac = int(meta["r_cnt"][c]), int(meta["a_cnt"][c])
        if rc == 0 or ac == 0:
            continue
        blk = res.results[c]["out"].reshape(Ro, Ap, H)
        r0, a0 = int(meta["r0"][c]), int(meta["a0"][c])
        out[r0:r0 + rc, a0:a0 + ac, :] = blk[:rc, :ac, :].astype(np.float32)
    return out
